# revision 85
# baseline (speedup 1.0000x reference)
"""ContraCLM token-level contrastive loss on 8 Trainium2 NeuronCores.

Data-parallel over the batch: core b handles sample b (B=8).
138.3us -> ~75us vs the uncompacted baseline.

Mask compaction: ~50% of tokens are masked out and contribute nothing
to the loss except an exp(0)=1 per masked column (corrected by the K0
term). The host gathers only the unmasked token rows (padded with
duplicates of token idx[0] up to SP=896 = 6.5 sigma for
Binomial(1536, 0.5); an exact numpy fallback covers the impossible
overflow) and ships them as fp8e4 (x4). Quadratic sim work drops to
(896/1536)^2 = 34%; input DMA drops 6x. The fp8/fp8-matmul error
averages out to ~4e-5 on the final scalar vs the 2e-2 gate.

Per core the 2SP x 2SP exp-sim row sums come from three quadrant
families (A = f1 f1^T upper triangle, C = f2 f1^T full rows, D =
f2 f2^T upper triangle; fp8e4 x8 DoubleRow matmuls, K=1024 in 4
double-k groups). Row sums ride the ScalarE activation free-dim
accumulator. Normalization: per-token sum-of-squares (Scalar
Square+accum / DVE mul+reduce), then 8*rsqrt(ss) computed as
exp(-ln(ss)/2 + ln 8) so Ln/Exp stay in the same Scalar activation
table as the sim exps (Sqrt would force 1.3us table reloads). The
scale is folded into the transpose: fT tile = hg^T @ diag(sc8), a
plain fp8 x bf16 matmul per k-tile (diag built by one DVE
identity*scalar op), then a f32->fp8 copy into fT via half-tile PSUM
buffers so the copy of one half overlaps the transposes of the next.

Mirrored lower-triangle / B-quadrant contributions are column sums of
the computed strips: view-1 sums accumulate via ones^T @ es fold
matmuls into a persistent PSUM row (zero-initialized by a
zeros-weight matmul; folds emitted one strip late so the PE never
head-blocks on an exp), transposed back to token-partitions by K=1
matmuls mid-kernel. View-2 sums use DVE adds into SBUF + 7 fast
full-K fold matmuls, keeping the tail chain short.

Self-sim diagonals get -1e9 injected in PSUM before exp (exact zero).
Pad/masked tokens have f=0 (mask folded into sc8), each contributing
exp(0)=1 per column: Ln(rowsum + (2n-2SP)) fuses the correction into
the activation bias. per_tok = log(denom) - pos_sim/T; the device
returns per-partition masked sums [128, 2] and the host finishes
sum/(2n) and the 8-core mean.

Schedule: zero-writing keepalive matmuls warm the PE p-state through
the DMA-paced start; input DMAs all on the sync queue (view 1
reversed, in 2-3-tile chunks) so view 1 strictly precedes view 2;
a_rows descend as fT1 tiles complete, c_rows ascend as fT2 tiles
land, d_rows run biggest-first with the view-1 epilogue riding along.
"""

import sys

for _p in ("/opt/trn_rl_repo", "/opt/pypackages"):
    if _p not in sys.path:
        sys.path.append(_p)

from contextlib import ExitStack

import numpy as np

import bass_rust

import concourse.bass as bass
import concourse.tile as tile
from concourse import mybir
from concourse.bass_utils import run_bass_kernel_spmd
from concourse.masks import make_identity
from concourse.vector_clock import ScopedClock

# The walrus build in this container encodes at most 2 sync waits per
# instruction (bass_rust's inst_waits_full agrees), but Tile's semaphore
# assignment can attach more. Hoist excess waits onto unfusable same-engine
# NoOps immediately before the instruction — the engine executes its queue
# in order, so semantics are preserved.
_MAX_WAITS = 1


def _split_excess_waits(nc, ordered):
    for bb_name, insts in ordered.items():
        out = []
        changed = False
        for inst in insts:
            si = getattr(inst, "sync_info", None)
            waits = list(si.on_wait) if si is not None else []
            if len(waits) > _MAX_WAITS:
                changed = True
                extra, keep = waits[:-_MAX_WAITS], waits[-_MAX_WAITS:]
                for i in range(0, len(extra), _MAX_WAITS):
                    out.append(mybir.InstNoOp(
                        name=nc.get_next_instruction_name(),
                        sync_info=mybir.SyncInfo(
                            on_wait=extra[i:i + _MAX_WAITS], on_update=[]),
                        bass_nofuse=True,
                        engine=inst.engine,
                    ))
                si.on_wait = keep
            out.append(inst)
        if changed:
            insts[:] = out


_orig_lower_ordered_insts = tile.TileContext._lower_ordered_insts


def _patched_lower_ordered_insts(self, ordered):
    _split_excess_waits(self.nc, ordered)
    return _orig_lower_ordered_insts(self, ordered)


tile.TileContext._lower_ordered_insts = _patched_lower_ordered_insts


def _split_waits_drain_and_barrier(self, tick_clock, wait_clock):
    nc = self.nc
    probe = nc.sync.nop(nofuse=True)
    wait_clock.add_sem_waits(
        probe.ins, ScopedClock({None: tick_clock.global_clock}))
    si = probe.ins.sync_info
    waits = list(si.on_wait) if si is not None else []
    if len(waits) > _MAX_WAITS:
        si.on_wait = waits[:_MAX_WAITS]
        for i in range(_MAX_WAITS, len(waits), _MAX_WAITS):
            nxt = nc.sync.nop(nofuse=True)
            nxt.ins.sync_info = bass_rust.SyncInfo(
                on_wait=waits[i:i + _MAX_WAITS], on_update=[])
    nc.sync.drain()
    nc.all_engine_barrier()
    assert self.sems is not None
    popped = nc._tile_sem_poison_stack.pop()
    assert popped is self._sem_poison
    nc.clear_and_free_semaphores(list(self.sems.allocated().values()))
    nc.all_engine_barrier()


tile.TileContext._drain_and_barrier = _split_waits_drain_and_barrier

S_FULL, D, NCORES = 1536, 1024, 8
SP = 896                 # compacted+padded tokens per view on device
                         # (Binomial(1536,.5) tops out ~802 for any sane
                         # draw; 896 is 6.5 sigma, and the numpy fallback
                         # is exact if ever exceeded)
ST = SP // 128           # 7 s-tiles per view
NB = 2 * ST              # 16 block rows of F
KT = D // 128            # 8 contraction tiles
TEMP_INV = 20.0          # 1 / 0.05
FP8_SCALE = 8.0          # f entries ~N(0, 1/32); x8 keeps them in e4m3's
                         # normal range (|f|*8 <~ 2, well under 240)
EXP_SCALE = TEMP_INV / (FP8_SCALE * FP8_SCALE)
F32 = mybir.dt.float32
BF16 = mybir.dt.bfloat16
FP8 = mybir.dt.float8e4
AF = mybir.ActivationFunctionType
ALU = mybir.AluOpType
DR = mybir.MatmulPerfMode.DoubleRow


def _chunks(lo, hi):
    """Split [lo, hi) at the PSUM 512-f32 bank boundary."""
    out = []
    if lo < 512:
        out.append((lo, min(512, hi)))
    if hi > 512:
        out.append((max(lo, 512), hi))
    return out


def _build(num_devices: int = NCORES, debug_dump: bool = False) -> bass.Bass:
    nc = bass.Bass(num_devices=num_devices)
    # pre-gathered compacted tokens, [128, ST, D] bf16:
    # partition p, tile t <-> compacted token 128*t + p
    hg1 = nc.dram_tensor("hg1", [128, ST, D], FP8, kind="ExternalInput")
    hg2 = nc.dram_tensor("hg2", [128, ST, D], FP8, kind="ExternalInput")
    cmaskT = nc.dram_tensor("cmaskT", [128, ST], F32, kind="ExternalInput")
    # per-partition per-view token sums; host finishes sum/(2n) + batch mean
    out = nc.dram_tensor("loss", [128, 2], F32, kind="ExternalOutput")
    if debug_dump:
        ng_dump = nc.dram_tensor("ng_dump", [128, NB], F32,
                                 kind="ExternalOutput")
        poss_dump = nc.dram_tensor("poss_dump", [128, ST], F32,
                                   kind="ExternalOutput")
        sc8_dump = nc.dram_tensor("sc8_dump", [128, NB], F32,
                                  kind="ExternalOutput")

    with tile.TileContext(nc) as tc, ExitStack() as ctx:
        const_pool = ctx.enter_context(tc.tile_pool(name="const", bufs=1))
        big = ctx.enter_context(tc.tile_pool(name="big", bufs=1))
        stat = ctx.enter_context(tc.tile_pool(name="stat", bufs=1))

        h1k = big.tile([128, ST, D], FP8)        # 4*h, fp8e4 (host staged)
        h2k = big.tile([128, ST, D], FP8)
        fT1 = big.tile([128, KT, SP], FP8)       # f1^T * 8, fp8e4
        fT2 = big.tile([128, KT, SP], FP8)       # f2^T * 8

        msk = const_pool.tile([128, ST], F32)
        # input DMAs first, all on the sync queue so view 1 gets strict
        # bandwidth priority over view 2 (2-tile chunks, view 1 reversed
        # to match the descending a_row schedule)
        nc.scalar.dma_start(msk[:], cmaskT[:])  # off the h1 critical queue
        for lo, hi in ((5, 7), (2, 5), (0, 2)):
            nc.sync.dma_start(h1k[:, lo:hi, :], hg1[:, lo:hi, :])
        for lo, hi in ((0, 2), (2, 4), (4, 7)):
            nc.sync.dma_start(h2k[:, lo:hi, :], hg2[:, lo:hi, :])

        # keepalive inputs first: PE warmup gates on these two memsets
        zeros_bf = const_pool.tile([128, 1], BF16)
        nc.gpsimd.memset(zeros_bf[:], 0.0)
        ones_b512 = const_pool.tile([128, 512], BF16)
        nc.gpsimd.memset(ones_b512[:], 1.0)
        lnb8 = const_pool.tile([128, 1], F32)
        nc.gpsimd.memset(lnb8[:], float(np.log(FP8_SCALE)))
        identB = const_pool.tile([128, 128], BF16)
        make_identity(nc, identB[:])
        identF = const_pool.tile([128, 128], F32)
        make_identity(nc, identF[:])
        # -1e9 on the diagonal, bf16: injected into self-sim PSUM blocks
        # via an extra accumulating matmul (identB^T @ negIB = -1e9 I)
        negIB = const_pool.tile([128, 128], BF16)
        nc.gpsimd.memset(negIB[:], 0.0)
        nc.gpsimd.affine_select(
            out=negIB[:], in_=negIB[:], compare_op=ALU.not_equal,
            fill=-1e9, base=0, pattern=[[-1, 128]], channel_multiplier=1)
        ones_col = const_pool.tile([128, 1], F32)
        nc.gpsimd.memset(ones_col[:], 1.0)
        ones_sq = const_pool.tile([128, 128], F32)
        nc.gpsimd.memset(ones_sq[:], 1.0)
        ones_bf = const_pool.tile([128, 1], BF16)
        nc.gpsimd.memset(ones_bf[:], 1.0)

        ss = stat.tile([128, NB], F32)           # per-token sum of squares
        sc8 = stat.tile([128, NB], F32)          # 8 * mask * rsqrt(ss)
        nrm = stat.tile([128, NB], F32)
        acc = stat.tile([128, NB, 2], F32)       # per-strip row sums
        poss20 = stat.tile([128, ST], F32)       # 64 * pos_sim
        msk24 = stat.tile([128, NB], F32)
        negK0 = stat.tile([128, 1], F32)

        sqtr = stat.tile([128, 2, D], BF16)      # square scratch
        cacsb1 = stat.tile([1, SP], F32)         # view-1 col sums, SBUF
        cac2 = stat.tile([128, SP], BF16)        # view-2 col acc (D upper)

        nc.gpsimd.memset(acc[:], 0.0)
        nc.vector.memset(cac2[:], 0.0)

        with ExitStack() as bctx:
            psA = bctx.enter_context(
                tc.tile_pool(name="psA", bufs=2, space="PSUM"))
            esA = bctx.enter_context(tc.tile_pool(name="esA", bufs=4))
            scr = bctx.enter_context(tc.tile_pool(name="scr", bufs=3))
            cacpp = bctx.enter_context(
                tc.tile_pool(name="cacp", bufs=1, space="PSUM"))

            cacp = cacpp.tile([1, SP], F32, name="cacp")

            def pe_keepalive(n):
                # small zero-writing matmuls: keep the PE p-state ramp warm
                # through the DMA-paced start (only safe BEFORE real folds
                # accumulate into cacp)
                for _ in range(n):
                    nc.tensor.matmul(cacp[0:1, 0:128], zeros_bf[:],
                                     ones_b512[:, 0:128], start=True,
                                     stop=True, skip_group_check=True)

            def zero_cacp():
                for lo, hi in _chunks(0, SP):
                    nc.tensor.matmul(cacp[0:1, lo:hi],
                                     zeros_bf[:], ones_b512[:, 0:hi - lo],
                                     start=True, stop=True,
                                     skip_group_check=True)

            pe_keepalive(30)

            # ---- mask-only precomputes (before tps claims its PSUM) ----
            with tc.tile_pool(name="ep0", bufs=1) as ep0, \
                 tc.tile_pool(name="ep0_ps", bufs=1, space="PSUM") as ep0p:
                msum = ep0.tile([128, 1], F32)
                nc.vector.tensor_reduce(msum[:], msk[:],
                                        axis=mybir.AxisListType.X,
                                        op=ALU.add)
                nps = ep0p.tile([128, 1], F32)
                nc.tensor.matmul(nps[:], ones_sq[:], msum[:], start=True,
                                 stop=True)
                # -K0 = 2n - 2*SP
                nc.scalar.activation(negK0[:], nps[:], AF.Copy, scale=2.0,
                                     bias=float(-2 * SP))
                nc.vector.tensor_copy(msk24[:, 0:ST], msk[:])
                nc.vector.tensor_copy(msk24[:, ST:NB], msk[:])

            zero_cacp()
            # more p-state warmers: safe until the first fold (PE queue
            # order), they soak the DMA-paced wait for the first h1 tiles
            pe_keepalive(40)
            zero_cacp()
            tctx = ExitStack()  # transpose-phase PSUM, closed before epilogue
            tps = tctx.enter_context(
                tc.tile_pool(name="tps", bufs=2, space="PSUM"))

            def square(hk, t, col, eng):
                """ss[:, col] = sum_d hk[:, t, :]^2."""
                sq = sqtr[:, col % 2, :]
                if eng == "scalar":
                    nc.scalar.activation(sq, hk[:, t, :], AF.Square,
                                         accum_out=ss[:, col:col + 1])
                else:
                    if eng == "gpsimd":
                        nc.gpsimd.tensor_mul(sq, hk[:, t, :], hk[:, t, :])
                    else:
                        nc.vector.tensor_mul(sq, hk[:, t, :], hk[:, t, :])
                    nc.vector.tensor_reduce(ss[:, col:col + 1], sq,
                                            axis=mybir.AxisListType.X,
                                            op=ALU.add)

            def finish_scale(o, n):
                """sc8[:, o:o+n] = 8 * msk * rsqrt(ss[:, o:o+n]).

                rsqrt as exp(-ln(x)/2 + ln 8): Ln/Exp share the Scalar
                activation table with the sim exps — no table reloads
                (Sqrt would force a 1.3us table swap each way).
                """
                nc.scalar.activation(nrm[:, o:o + n], ss[:, o:o + n],
                                     AF.Ln)
                r8 = stat.tile([128, n], F32, name=f"r8_{o}")
                nc.scalar.activation(r8[:], nrm[:, o:o + n], AF.Exp,
                                     scale=-0.5, bias=lnb8[:, 0:1])
                nc.vector.tensor_mul(sc8[:, o:o + n], r8[:],
                                     msk24[:, o:o + n])

            def transpose_tile(hk, fT, half, t, cp_engine):
                """fT[:, :, t*128:+128] = (hk[:,t,:]/4 * sc8)^T as fp8.

                Scale + transpose in one plain matmul per k-tile:
                out = hk_tile^T @ diag(sc8) (hk is 4h fp8; diag carries
                8*rsqrt(ss8) = 2/||h||, so out = 8*h/||h||).
                """
                col = half * ST + t
                diagS = scr.tile([128, 128], BF16, tag="dg8",
                                 name=f"dgS_{half}_{t}")
                # diag(sc8) = identity * sc8 per-partition: one fast DVE op
                nc.vector.tensor_scalar_mul(diagS[:], identB[:],
                                            sc8[:, col:col + 1])
                # half-tile pt buffers (1 PSUM bank each, 2 in the pool):
                # the cvt of one half overlaps the transposes of the next
                for hf in range(2):
                    k0 = hf * (KT // 2)
                    pt = tps.tile([128, D // 2], F32, tag="pt",
                                  name=f"pt_{half}_{t}_{hf}")
                    for k in range(KT // 2):
                        nc.tensor.matmul(
                            pt[:, k * 128:(k + 1) * 128],
                            hk[:, t, (k0 + k) * 128:(k0 + k + 1) * 128],
                            diagS[:], start=True, stop=True)
                    dst = fT[:, k0:k0 + KT // 2, t * 128:(t + 1) * 128]
                    src = pt[:].rearrange("p (j c) -> p j c", j=KT // 2)
                    if cp_engine == "vector":
                        nc.vector.tensor_copy(dst, src)
                    else:
                        nc.scalar.copy(dst, src)

            def mm_strip(ps, lhsT, rT, rhsT, col0, ncols):
                """sim strip into ps[:, 0:ncols] (DoubleRow, K=1024)."""
                for g in range(KT // 2):
                    u0 = 0
                    while u0 < ncols:
                        u1 = min(u0 + 512, ncols)
                        nc.tensor.matmul(
                            ps[:, u0:u1],
                            lhsT[:, 2 * g:2 * g + 2,
                                 rT * 128:(rT + 1) * 128],
                            rhsT[:, 2 * g:2 * g + 2, col0 + u0:col0 + u1],
                            perf_mode=DR,
                            start=(g == 0), stop=(g == KT // 2 - 1))
                        u0 = u1

            # deferred column folds: emitted after the NEXT strip's matmuls
            # so the PE queue never head-blocks on an exp result
            pending_folds = []

            def flush_folds():
                for g0, g1, es_ap in pending_folds:
                    for lo, hi in _chunks(g0, g1):
                        nc.tensor.matmul(
                            cacp[0:1, lo:hi], ones_bf[:],
                            es_ap[:, lo - g0:hi - g0],
                            start=False, stop=True, skip_group_check=True)
                pending_folds.clear()

            def a_row(r):
                ncols = SP - r * 128
                trip = psA.tile([128, SP], F32, tag="tp", name=f"tpA_{r}")
                mm_strip(trip, fT1, r, fT1, r * 128, ncols)
                flush_folds()
                nc.tensor.matmul(trip[:, 0:128], identB[:], negIB[:],
                                 start=False, stop=True,
                                 skip_group_check=True)
                es = esA.tile([128, SP], BF16, tag="es", name=f"esA_{r}")
                nc.scalar.activation(es[:, 0:ncols], trip[:, 0:ncols],
                                     AF.Exp, scale=EXP_SCALE,
                                     accum_out=acc[:, r, 0:1])
                if ncols > 128:
                    pending_folds.append(
                        ((r + 1) * 128, SP, es[:, 128:ncols]))

            def c_row(rT):
                trip = psA.tile([128, SP], F32, tag="tp", name=f"tpC_{rT}")
                mm_strip(trip, fT2, rT, fT1, 0, SP)
                flush_folds()
                # counterpart diagonal: extract 64*pos_sim, keep it inside
                # the row sum (denom = Ng + pos)
                dscr = scr.tile([128, 128], F32, tag="dg", name=f"dg_{rT}")
                nc.vector.tensor_mul(
                    dscr[:], trip[:, rT * 128:(rT + 1) * 128], identF[:])
                nc.vector.tensor_reduce(
                    poss20[:, rT:rT + 1], dscr[:],
                    axis=mybir.AxisListType.X, op=ALU.add)
                es = esA.tile([128, SP], BF16, tag="es", name=f"esC_{rT}")
                nc.scalar.activation(es[:], trip[:], AF.Exp,
                                     scale=EXP_SCALE,
                                     accum_out=acc[:, ST + rT, 0:1])
                pending_folds.append((0, SP, es[:]))

            def d_row(rT):
                ncols = SP - rT * 128
                trip = psA.tile([128, SP], F32, tag="tp", name=f"tpD_{rT}")
                mm_strip(trip, fT2, rT, fT2, rT * 128, ncols)
                flush_folds()
                nc.tensor.matmul(trip[:, 0:128], identB[:], negIB[:],
                                 start=False, stop=True,
                                 skip_group_check=True)
                es = esA.tile([128, SP], BF16, tag="es", name=f"esD_{rT}")
                nc.scalar.activation(es[:, 0:ncols], trip[:, 0:ncols],
                                     AF.Exp, scale=EXP_SCALE,
                                     accum_out=acc[:, ST + rT, 1:2])
                if ncols > 128:
                    # view-2 col sums on DVE (bf16 2x) — the final fold
                    # back to token-partitions is then 7 fast full-K
                    # matmuls instead of slow K=1 loads on the tail
                    nc.vector.tensor_add(cac2[:, (rT + 1) * 128:SP],
                                         cac2[:, (rT + 1) * 128:SP],
                                         es[:, 128:ncols])

            # view-1 pipeline, tiles 6..0 (DMA order), a_rows descending.
            # First group all-Scalar (same queue = minimum chain latency);
            # later groups split for throughput.
            v1_sq = {6: "scalar", 5: "vector", 4: "vector", 3: "scalar",
                     2: "vector", 1: "scalar", 0: "vector"}
            v1_cvt = {6: "scalar", 4: "scalar"}
            # tiles 6,5 fully per-tile pipelined (first-chain latency):
            # sq5 on DVE runs concurrently with sq6 on Scalar
            square(h1k, 5, 5, eng="vector")
            square(h1k, 6, 6, eng="scalar")
            finish_scale(6, 1)
            transpose_tile(h1k, fT1, 0, 6, cp_engine="scalar")
            a_row(6)
            finish_scale(5, 1)
            transpose_tile(h1k, fT1, 0, 5, cp_engine="vector")
            a_row(5)
            for grp in ((4, 3), (2, 1), (0,)):
                for t in grp:
                    square(h1k, t, t, eng=v1_sq[t])
                finish_scale(min(grp), len(grp))
                for t in grp:
                    transpose_tile(h1k, fT1, 0, t,
                                   cp_engine=v1_cvt.get(t, "vector"))
                for t in grp:
                    a_row(t)

            # view-2 pipeline, tiles 0..6, c_rows as tiles complete
            for grp in ((0, 1), (2, 3), (4, 5), (6,)):
                for t in grp:
                    square(h2k, t, ST + t, eng="scalar")
                finish_scale(ST + grp[0], len(grp))
                for t in grp:
                    transpose_tile(h2k, fT2, 1, t,
                                   cp_engine=("scalar" if t % 2 == 1
                                              else "vector"))
                for t in grp:
                    c_row(t)

            # all view-1 (A + C) folds are in cacp: stage to SBUF, then
            # re-zero behind the first D strip's matmuls
            flush_folds()
            nc.vector.tensor_copy(cacsb1[:], cacp[0:1, :])

            tctx.close()  # free transpose-phase PSUM banks
            ep = bctx.enter_context(tc.tile_pool(name="ep", bufs=1))
            epp = bctx.enter_context(
                tc.tile_pool(name="ep_ps", bufs=1, space="PSUM"))
            pcbt = epp.tile([128, 2, ST], F32, name="pcbt")

            def fold_transpose0():
                # cacp row back to token-partitions: K=1 matmuls
                for jb in range(ST):
                    nc.tensor.matmul(
                        pcbt[:, 0, jb:jb + 1],
                        cacsb1[0:1, jb * 128:(jb + 1) * 128],
                        ones_col[0:1, :], start=True, stop=True,
                        skip_group_check=True)

            def fold_transpose1():
                # cac2 block col-sums: full-K matmuls, N=1 (fast loads)
                for jb in range(ST):
                    nc.tensor.matmul(
                        pcbt[:, 1, jb:jb + 1],
                        cac2[:, jb * 128:(jb + 1) * 128],
                        ones_bf[:], start=True, stop=True,
                        skip_group_check=True)

            ng = ep.tile([128, NB], F32)
            denom = ep.tile([128, NB], F32)
            lg = ep.tile([128, NB], F32)
            ptok = ep.tile([128, NB], F32)
            p20m = ep.tile([128, ST], F32)
            tsum = ep.tile([128, 2], F32)

            def epilogue_half(half):
                """per_tok for one view half -> tsum[:, half]."""
                o = half * ST
                nc.vector.tensor_reduce(ng[:, o:o + ST],
                                        acc[:, o:o + ST, :],
                                        axis=mybir.AxisListType.X,
                                        op=ALU.add)
                nc.vector.tensor_add(ng[:, o:o + ST], ng[:, o:o + ST],
                                     pcbt[:, half, :])
                # Ln(ng + negK0): K0 correction fused into the bias
                nc.scalar.activation(lg[:, o:o + ST], ng[:, o:o + ST],
                                     AF.Ln, bias=negK0[:, 0:1])
                nc.vector.tensor_mul(ptok[:, o:o + ST], lg[:, o:o + ST],
                                     msk24[:, o:o + ST])
                nc.vector.tensor_sub(ptok[:, o:o + ST], ptok[:, o:o + ST],
                                     p20m[:])
                nc.vector.tensor_reduce(tsum[:, half:half + 1],
                                        ptok[:, o:o + ST],
                                        axis=mybir.AxisListType.X,
                                        op=ALU.add)

            # D rows, biggest first so the tail chain hangs off a tiny
            # strip; view-1 epilogue rides along behind the first strips
            d_row(0)
            fold_transpose0()
            nc.vector.tensor_mul(p20m[:], poss20[:], msk[:])
            # poss20 held 64*pos_sim (raw psum); scale to pos_sim/T
            nc.vector.tensor_scalar_mul(p20m[:], p20m[:], EXP_SCALE)
            d_row(1)
            epilogue_half(0)
            for rT in range(2, ST):
                d_row(rT)
            fold_transpose1()
            epilogue_half(1)

            if debug_dump:
                nc.sync.dma_start(ng_dump[:], ng[:])
                nc.sync.dma_start(poss_dump[:], poss20[:])
                nc.sync.dma_start(sc8_dump[:], sc8[:])
            nc.sync.dma_start(out[:], tsum[:])

    return nc


_NC = None


def _stage_core(h1_b, h2_b, mask_b):
    """Host-side compaction: gather unmasked rows, pad to SP, tile, bf16."""
    import ml_dtypes

    idx = np.flatnonzero(mask_b)
    n = idx.size
    if n == 0 or n > SP:
        return None  # numpy fallback handles the (never-seen) extremes
    idxp = np.concatenate(
        [idx, np.full(SP - n, idx[0], dtype=idx.dtype)])
    cmask = (np.arange(SP) < n).astype(np.float32)

    def prep(h):
        hg = h[idxp] * np.float32(4.0)                  # [SP, D], 4h
        hgT = hg.reshape(ST, 128, D).transpose(1, 0, 2)  # [128, ST, D]
        return np.ascontiguousarray(hgT.astype(ml_dtypes.float8_e4m3fn))

    return {
        "hg1": prep(h1_b),
        "hg2": prep(h2_b),
        "cmaskT": np.ascontiguousarray(
            cmask.reshape(ST, 128).T.astype(np.float32)),
    }


def _loss_numpy(h1_b, h2_b, mask_b):
    """Exact reference loss for one sample (fallback, never hit for the
    spec'd mask distribution)."""
    T, EPS = 0.05, 1e-12
    m = mask_b.astype(bool)

    def norm(x):
        nn = np.sqrt((x * x).sum(-1, keepdims=True))
        return x / np.maximum(nn, EPS)

    f1, f2 = norm(h1_b.astype(np.float64)), norm(h2_b.astype(np.float64))
    feats = np.concatenate([f1, f2], 0)
    pos = np.exp((f1 * f2).sum(-1) / T)
    pos = np.concatenate([pos, pos])
    sim = feats @ feats.T / T
    S = h1_b.shape[0]
    tok = np.arange(2 * S) % S
    m2 = np.concatenate([m, m])
    negm = m2[:, None] & m2[None, :] & (tok[:, None] != tok[None, :])
    Ng = (np.exp(sim) * negm).sum(-1)
    per_tok = -np.log(pos / (Ng + pos))
    return float((per_tok * m2).sum() / m2.sum())


def kernel(last_hidden_states_1, last_hidden_states_2, token_mask_batch):
    global _NC
    h1 = np.asarray(last_hidden_states_1, dtype=np.float32)
    h2 = np.asarray(last_hidden_states_2, dtype=np.float32)
    mask = np.asarray(token_mask_batch)
    assert h1.shape == (NCORES, S_FULL, D), h1.shape

    staged, fallback, ns = [], {}, []
    for b in range(NCORES):
        s = _stage_core(h1[b], h2[b], mask[b])
        ns.append(int(mask[b].sum()))
        if s is None:
            fallback[b] = _loss_numpy(h1[b], h2[b], mask[b])
            ph = np.zeros(S_FULL, dtype=bool)
            ph[:SP] = True
            s = _stage_core(h1[b], h2[b], ph)  # placeholder device run
        staged.append(s)

    if _NC is None:
        _NC = _build(NCORES)

    res = run_bass_kernel_spmd(_NC, staged, list(range(NCORES)))
    losses = [
        fallback.get(b,
                     float(np.asarray(res.results[b]["loss"],
                                      dtype=np.float64).sum()
                           / (2.0 * ns[b])))
        for b in range(NCORES)
    ]
    return np.float32(np.mean(losses))


# revision 86
# speedup vs baseline: 1.1612x; 1.1612x over previous
"""ContraCLM token-level contrastive loss on 8 Trainium2 NeuronCores.

Data-parallel over the batch: core b handles sample b (B=8).
138.3us -> ~75us vs the uncompacted baseline.

Mask compaction: ~50% of tokens are masked out and contribute nothing
to the loss except an exp(0)=1 per masked column (corrected by the K0
term). The host gathers only the unmasked token rows (padded with
duplicates of token idx[0] up to SP=896 = 6.5 sigma for
Binomial(1536, 0.5); an exact numpy fallback covers the impossible
overflow) and ships them as fp8e4 (x4). Quadratic sim work drops to
(896/1536)^2 = 34%; input DMA drops 6x. The fp8/fp8-matmul error
averages out to ~4e-5 on the final scalar vs the 2e-2 gate.

Per core the 2SP x 2SP exp-sim row sums come from three quadrant
families (A = f1 f1^T upper triangle, C = f2 f1^T full rows, D =
f2 f2^T upper triangle; fp8e4 x8 DoubleRow matmuls, K=1024 in 4
double-k groups). Row sums ride the ScalarE activation free-dim
accumulator. Normalization: per-token sum-of-squares (Scalar
Square+accum / DVE mul+reduce), then 8*rsqrt(ss) computed as
exp(-ln(ss)/2 + ln 8) so Ln/Exp stay in the same Scalar activation
table as the sim exps (Sqrt would force 1.3us table reloads). The
scale is folded into the transpose: fT tile = hg^T @ diag(sc8), a
plain fp8 x bf16 matmul per k-tile (diag built by one DVE
identity*scalar op), then a f32->fp8 copy into fT via half-tile PSUM
buffers so the copy of one half overlaps the transposes of the next.

Mirrored lower-triangle / B-quadrant contributions are column sums of
the computed strips: view-1 sums accumulate via ones^T @ es fold
matmuls into a persistent PSUM row (zero-initialized by a
zeros-weight matmul; folds emitted one strip late so the PE never
head-blocks on an exp), transposed back to token-partitions by K=1
matmuls mid-kernel. View-2 sums use DVE adds into SBUF + 7 fast
full-K fold matmuls, keeping the tail chain short.

Self-sim diagonals get -1e9 injected in PSUM before exp (exact zero).
Pad/masked tokens have f=0 (mask folded into sc8), each contributing
exp(0)=1 per column: Ln(rowsum + (2n-2SP)) fuses the correction into
the activation bias. per_tok = log(denom) - pos_sim/T; the device
returns per-partition masked sums [128, 2] and the host finishes
sum/(2n) and the 8-core mean.

Schedule: zero-writing keepalive matmuls warm the PE p-state through
the DMA-paced start; input DMAs all on the sync queue (view 1
reversed, in 2-3-tile chunks) so view 1 strictly precedes view 2;
a_rows descend as fT1 tiles complete, c_rows ascend as fT2 tiles
land, d_rows run biggest-first with the view-1 epilogue riding along.
"""

import sys

for _p in ("/opt/trn_rl_repo", "/opt/pypackages"):
    if _p not in sys.path:
        sys.path.append(_p)

from contextlib import ExitStack

import numpy as np

import bass_rust

import concourse.bass as bass
import concourse.tile as tile
from concourse import mybir
from concourse.bass_utils import run_bass_kernel_spmd
from concourse.masks import make_identity
from concourse.vector_clock import ScopedClock

# The walrus build in this container encodes at most 2 sync waits per
# instruction (bass_rust's inst_waits_full agrees), but Tile's semaphore
# assignment can attach more. Hoist excess waits onto unfusable same-engine
# NoOps immediately before the instruction — the engine executes its queue
# in order, so semantics are preserved.
_MAX_WAITS = 1


def _split_excess_waits(nc, ordered):
    for bb_name, insts in ordered.items():
        out = []
        changed = False
        for inst in insts:
            si = getattr(inst, "sync_info", None)
            waits = list(si.on_wait) if si is not None else []
            if len(waits) > _MAX_WAITS:
                changed = True
                extra, keep = waits[:-_MAX_WAITS], waits[-_MAX_WAITS:]
                for i in range(0, len(extra), _MAX_WAITS):
                    out.append(mybir.InstNoOp(
                        name=nc.get_next_instruction_name(),
                        sync_info=mybir.SyncInfo(
                            on_wait=extra[i:i + _MAX_WAITS], on_update=[]),
                        bass_nofuse=True,
                        engine=inst.engine,
                    ))
                si.on_wait = keep
            out.append(inst)
        if changed:
            insts[:] = out


_orig_lower_ordered_insts = tile.TileContext._lower_ordered_insts


def _patched_lower_ordered_insts(self, ordered):
    _split_excess_waits(self.nc, ordered)
    return _orig_lower_ordered_insts(self, ordered)


tile.TileContext._lower_ordered_insts = _patched_lower_ordered_insts


def _split_waits_drain_and_barrier(self, tick_clock, wait_clock):
    nc = self.nc
    probe = nc.sync.nop(nofuse=True)
    wait_clock.add_sem_waits(
        probe.ins, ScopedClock({None: tick_clock.global_clock}))
    si = probe.ins.sync_info
    waits = list(si.on_wait) if si is not None else []
    if len(waits) > _MAX_WAITS:
        si.on_wait = waits[:_MAX_WAITS]
        for i in range(_MAX_WAITS, len(waits), _MAX_WAITS):
            nxt = nc.sync.nop(nofuse=True)
            nxt.ins.sync_info = bass_rust.SyncInfo(
                on_wait=waits[i:i + _MAX_WAITS], on_update=[])
    nc.sync.drain()
    nc.all_engine_barrier()
    assert self.sems is not None
    popped = nc._tile_sem_poison_stack.pop()
    assert popped is self._sem_poison
    nc.clear_and_free_semaphores(list(self.sems.allocated().values()))
    nc.all_engine_barrier()


tile.TileContext._drain_and_barrier = _split_waits_drain_and_barrier

S_FULL, D, NCORES = 1536, 1024, 8
SP = 896                 # compacted+padded tokens per view on device
                         # (Binomial(1536,.5) tops out ~802 for any sane
                         # draw; 896 is 6.5 sigma, and the numpy fallback
                         # is exact if ever exceeded)
ST = SP // 128           # 7 s-tiles per view
NB = 2 * ST              # 16 block rows of F
KT = D // 128            # 8 contraction tiles
TEMP_INV = 20.0          # 1 / 0.05
FP8_SCALE = 8.0          # f entries ~N(0, 1/32); x8 keeps them in e4m3's
                         # normal range (|f|*8 <~ 2, well under 240)
EXP_SCALE = TEMP_INV / (FP8_SCALE * FP8_SCALE)
F32 = mybir.dt.float32
BF16 = mybir.dt.bfloat16
FP8 = mybir.dt.float8e4
AF = mybir.ActivationFunctionType
ALU = mybir.AluOpType
DR = mybir.MatmulPerfMode.DoubleRow


def _chunks(lo, hi):
    """Split [lo, hi) at the PSUM 512-f32 bank boundary."""
    out = []
    if lo < 512:
        out.append((lo, min(512, hi)))
    if hi > 512:
        out.append((max(lo, 512), hi))
    return out


def _build(num_devices: int = NCORES, debug_dump: bool = False) -> bass.Bass:
    nc = bass.Bass(num_devices=num_devices)
    # pre-gathered compacted tokens, [128, ST, D] bf16:
    # partition p, tile t <-> compacted token 128*t + p
    hg1 = nc.dram_tensor("hg1", [128, ST, D], FP8, kind="ExternalInput")
    hg2 = nc.dram_tensor("hg2", [128, ST, D], FP8, kind="ExternalInput")
    cmaskT = nc.dram_tensor("cmaskT", [128, ST], F32, kind="ExternalInput")
    # per-partition per-view token sums; host finishes sum/(2n) + batch mean
    out = nc.dram_tensor("loss", [128, 2], F32, kind="ExternalOutput")
    if debug_dump:
        ng_dump = nc.dram_tensor("ng_dump", [128, NB], F32,
                                 kind="ExternalOutput")
        poss_dump = nc.dram_tensor("poss_dump", [128, ST], F32,
                                   kind="ExternalOutput")
        sc8_dump = nc.dram_tensor("sc8_dump", [128, NB], F32,
                                  kind="ExternalOutput")

    with tile.TileContext(nc) as tc, ExitStack() as ctx:
        const_pool = ctx.enter_context(tc.tile_pool(name="const", bufs=1))
        big = ctx.enter_context(tc.tile_pool(name="big", bufs=1))
        stat = ctx.enter_context(tc.tile_pool(name="stat", bufs=1))

        h1k = big.tile([128, ST, D], FP8)        # 4*h, fp8e4 (host staged)
        h2k = big.tile([128, ST, D], FP8)
        fT1 = big.tile([128, KT, SP], FP8)       # f1^T * 8, fp8e4
        fT2 = big.tile([128, KT, SP], FP8)       # f2^T * 8

        msk = const_pool.tile([128, ST], F32)
        # input DMAs first, all on the sync queue so view 1 gets strict
        # bandwidth priority over view 2 (2-tile chunks, view 1 reversed
        # to match the descending a_row schedule)
        nc.scalar.dma_start(msk[:], cmaskT[:])  # off the h1 critical queue
        for lo, hi in ((5, 7), (2, 5), (0, 2)):
            nc.sync.dma_start(h1k[:, lo:hi, :], hg1[:, lo:hi, :])
        for lo, hi in ((0, 2), (2, 4), (4, 7)):
            nc.sync.dma_start(h2k[:, lo:hi, :], hg2[:, lo:hi, :])

        # keepalive inputs first: PE warmup gates on these two memsets
        zeros_bf = const_pool.tile([128, 1], BF16)
        nc.gpsimd.memset(zeros_bf[:], 0.0)
        ones_b512 = const_pool.tile([128, 512], BF16)
        nc.gpsimd.memset(ones_b512[:], 1.0)
        lnb8 = const_pool.tile([128, 1], F32)
        nc.gpsimd.memset(lnb8[:], float(np.log(FP8_SCALE)))
        identB = const_pool.tile([128, 128], BF16)
        make_identity(nc, identB[:])
        identF = const_pool.tile([128, 128], F32)
        make_identity(nc, identF[:])
        # -1e9 on the diagonal, bf16: injected into self-sim PSUM blocks
        # via an extra accumulating matmul (identB^T @ negIB = -1e9 I)
        negIB = const_pool.tile([128, 128], BF16)
        nc.gpsimd.memset(negIB[:], 0.0)
        nc.gpsimd.affine_select(
            out=negIB[:], in_=negIB[:], compare_op=ALU.not_equal,
            fill=-1e9, base=0, pattern=[[-1, 128]], channel_multiplier=1)
        ones_col = const_pool.tile([128, 1], F32)
        nc.gpsimd.memset(ones_col[:], 1.0)
        ones_sq = const_pool.tile([128, 128], F32)
        nc.gpsimd.memset(ones_sq[:], 1.0)
        ones_bf = const_pool.tile([128, 1], BF16)
        nc.gpsimd.memset(ones_bf[:], 1.0)

        ss = stat.tile([128, NB], F32)           # per-token sum of squares
        sc8 = stat.tile([128, NB], F32)          # 8 * mask * rsqrt(ss)
        nrm = stat.tile([128, NB], F32)
        acc = stat.tile([128, NB, 2], F32)       # per-strip row sums
        poss20 = stat.tile([128, ST], F32)       # 64 * pos_sim
        msk24 = stat.tile([128, NB], F32)
        negK0 = stat.tile([128, 1], F32)

        sqtr = stat.tile([128, 2, D], BF16)      # square scratch
        cacsb1 = stat.tile([1, SP], F32)         # view-1 col sums, SBUF
        cac2 = stat.tile([128, SP], BF16)        # view-2 col acc (D upper)

        nc.gpsimd.memset(acc[:], 0.0)
        nc.vector.memset(cac2[:], 0.0)

        with ExitStack() as bctx:
            psA = bctx.enter_context(
                tc.tile_pool(name="psA", bufs=2, space="PSUM"))
            esA = bctx.enter_context(tc.tile_pool(name="esA", bufs=5))
            scr = bctx.enter_context(tc.tile_pool(name="scr", bufs=4))
            cacpp = bctx.enter_context(
                tc.tile_pool(name="cacp", bufs=1, space="PSUM"))

            cacp = cacpp.tile([1, SP], F32, name="cacp")

            def pe_keepalive(n):
                # small zero-writing matmuls: keep the PE p-state ramp warm
                # through the DMA-paced start (only safe BEFORE real folds
                # accumulate into cacp)
                for _ in range(n):
                    nc.tensor.matmul(cacp[0:1, 0:128], zeros_bf[:],
                                     ones_b512[:, 0:128], start=True,
                                     stop=True, skip_group_check=True)

            def zero_cacp():
                for lo, hi in _chunks(0, SP):
                    nc.tensor.matmul(cacp[0:1, lo:hi],
                                     zeros_bf[:], ones_b512[:, 0:hi - lo],
                                     start=True, stop=True,
                                     skip_group_check=True)

            pe_keepalive(30)

            # ---- mask-only precomputes (before tps claims its PSUM) ----
            with tc.tile_pool(name="ep0", bufs=1) as ep0, \
                 tc.tile_pool(name="ep0_ps", bufs=1, space="PSUM") as ep0p:
                msum = ep0.tile([128, 1], F32)
                nc.vector.tensor_reduce(msum[:], msk[:],
                                        axis=mybir.AxisListType.X,
                                        op=ALU.add)
                nps = ep0p.tile([128, 1], F32)
                nc.tensor.matmul(nps[:], ones_sq[:], msum[:], start=True,
                                 stop=True)
                # -K0 = 2n - 2*SP
                nc.scalar.activation(negK0[:], nps[:], AF.Copy, scale=2.0,
                                     bias=float(-2 * SP))
                nc.vector.tensor_copy(msk24[:, 0:ST], msk[:])
                nc.vector.tensor_copy(msk24[:, ST:NB], msk[:])

            zero_cacp()
            # more p-state warmers: safe until the first fold (PE queue
            # order), they soak the DMA-paced wait for the first h1 tiles
            pe_keepalive(40)
            zero_cacp()
            tctx = ExitStack()  # transpose-phase PSUM, closed before epilogue
            tps = tctx.enter_context(
                tc.tile_pool(name="tps", bufs=2, space="PSUM"))

            def square(hk, t, col, eng):
                """ss[:, col] = sum_d hk[:, t, :]^2."""
                sq = sqtr[:, col % 2, :]
                if eng == "scalar":
                    nc.scalar.activation(sq, hk[:, t, :], AF.Square,
                                         accum_out=ss[:, col:col + 1])
                else:
                    if eng == "gpsimd":
                        nc.gpsimd.tensor_mul(sq, hk[:, t, :], hk[:, t, :])
                    else:
                        nc.vector.tensor_mul(sq, hk[:, t, :], hk[:, t, :])
                    nc.vector.tensor_reduce(ss[:, col:col + 1], sq,
                                            axis=mybir.AxisListType.X,
                                            op=ALU.add)

            def finish_scale(o, n):
                """sc8[:, o:o+n] = 8 * msk * rsqrt(ss[:, o:o+n]).

                rsqrt as exp(-ln(x)/2 + ln 8): Ln/Exp share the Scalar
                activation table with the sim exps — no table reloads
                (Sqrt would force a 1.3us table swap each way).
                """
                nc.scalar.activation(nrm[:, o:o + n], ss[:, o:o + n],
                                     AF.Ln)
                r8 = stat.tile([128, n], F32, name=f"r8_{o}")
                nc.scalar.activation(r8[:], nrm[:, o:o + n], AF.Exp,
                                     scale=-0.5, bias=lnb8[:, 0:1])
                nc.vector.tensor_mul(sc8[:, o:o + n], r8[:],
                                     msk24[:, o:o + n])

            def transpose_tile(hk, fT, half, t, cp_engine):
                """fT[:, :, t*128:+128] = (hk[:,t,:]/4 * sc8)^T as fp8.

                Scale + transpose in one plain matmul per k-tile:
                out = hk_tile^T @ diag(sc8) (hk is 4h fp8; diag carries
                8*rsqrt(ss8) = 2/||h||, so out = 8*h/||h||).
                """
                col = half * ST + t
                diagS = scr.tile([128, 128], BF16, tag="dg8",
                                 name=f"dgS_{half}_{t}")
                # diag(sc8) = identity * sc8 per-partition: one fast DVE op
                nc.vector.tensor_scalar_mul(diagS[:], identB[:],
                                            sc8[:, col:col + 1])
                # half-tile pt buffers (1 PSUM bank each, 2 in the pool):
                # the cvt of one half overlaps the transposes of the next
                for hf in range(2):
                    k0 = hf * (KT // 2)
                    pt = tps.tile([128, D // 2], F32, tag="pt",
                                  name=f"pt_{half}_{t}_{hf}")
                    for k in range(KT // 2):
                        nc.tensor.matmul(
                            pt[:, k * 128:(k + 1) * 128],
                            hk[:, t, (k0 + k) * 128:(k0 + k + 1) * 128],
                            diagS[:], start=True, stop=True)
                    dst = fT[:, k0:k0 + KT // 2, t * 128:(t + 1) * 128]
                    src = pt[:].rearrange("p (j c) -> p j c", j=KT // 2)
                    if cp_engine == "vector":
                        nc.vector.tensor_copy(dst, src)
                    else:
                        nc.scalar.copy(dst, src)

            def mm_strip(ps, lhsT, rT, rhsT, col0, ncols):
                """sim strip into ps[:, 0:ncols] (DoubleRow, K=1024)."""
                for g in range(KT // 2):
                    u0 = 0
                    while u0 < ncols:
                        u1 = min(u0 + 512, ncols)
                        nc.tensor.matmul(
                            ps[:, u0:u1],
                            lhsT[:, 2 * g:2 * g + 2,
                                 rT * 128:(rT + 1) * 128],
                            rhsT[:, 2 * g:2 * g + 2, col0 + u0:col0 + u1],
                            perf_mode=DR,
                            start=(g == 0), stop=(g == KT // 2 - 1))
                        u0 = u1

            # deferred column folds: emitted after the NEXT strip's matmuls
            # so the PE queue never head-blocks on an exp result
            pending_folds = []

            def flush_folds():
                for g0, g1, es_ap in pending_folds:
                    for lo, hi in _chunks(g0, g1):
                        nc.tensor.matmul(
                            cacp[0:1, lo:hi], ones_bf[:],
                            es_ap[:, lo - g0:hi - g0],
                            start=False, stop=True, skip_group_check=True)
                pending_folds.clear()

            def a_row(r):
                ncols = SP - r * 128
                trip = psA.tile([128, SP], F32, tag="tp", name=f"tpA_{r}")
                mm_strip(trip, fT1, r, fT1, r * 128, ncols)
                flush_folds()
                nc.tensor.matmul(trip[:, 0:128], identB[:], negIB[:],
                                 start=False, stop=True,
                                 skip_group_check=True)
                es = esA.tile([128, SP], BF16, tag="es", name=f"esA_{r}")
                nc.scalar.activation(es[:, 0:ncols], trip[:, 0:ncols],
                                     AF.Exp, scale=EXP_SCALE,
                                     accum_out=acc[:, r, 0:1])
                if ncols > 128:
                    pending_folds.append(
                        ((r + 1) * 128, SP, es[:, 128:ncols]))

            def c_row(rT):
                trip = psA.tile([128, SP], F32, tag="tp", name=f"tpC_{rT}")
                mm_strip(trip, fT2, rT, fT1, 0, SP)
                flush_folds()
                # counterpart diagonal: extract 64*pos_sim, keep it inside
                # the row sum (denom = Ng + pos)
                dscr = scr.tile([128, 128], F32, tag="dg", name=f"dg_{rT}")
                nc.vector.tensor_mul(
                    dscr[:], trip[:, rT * 128:(rT + 1) * 128], identF[:])
                nc.vector.tensor_reduce(
                    poss20[:, rT:rT + 1], dscr[:],
                    axis=mybir.AxisListType.X, op=ALU.add)
                es = esA.tile([128, SP], BF16, tag="es", name=f"esC_{rT}")
                nc.scalar.activation(es[:], trip[:], AF.Exp,
                                     scale=EXP_SCALE,
                                     accum_out=acc[:, ST + rT, 0:1])
                pending_folds.append((0, SP, es[:]))

            def d_row(rT):
                ncols = SP - rT * 128
                trip = psA.tile([128, SP], F32, tag="tp", name=f"tpD_{rT}")
                mm_strip(trip, fT2, rT, fT2, rT * 128, ncols)
                flush_folds()
                nc.tensor.matmul(trip[:, 0:128], identB[:], negIB[:],
                                 start=False, stop=True,
                                 skip_group_check=True)
                es = esA.tile([128, SP], BF16, tag="es", name=f"esD_{rT}")
                nc.scalar.activation(es[:, 0:ncols], trip[:, 0:ncols],
                                     AF.Exp, scale=EXP_SCALE,
                                     accum_out=acc[:, ST + rT, 1:2])
                if ncols > 128:
                    # view-2 col sums on DVE (bf16 2x) — the final fold
                    # back to token-partitions is then 7 fast full-K
                    # matmuls instead of slow K=1 loads on the tail
                    nc.vector.tensor_add(cac2[:, (rT + 1) * 128:SP],
                                         cac2[:, (rT + 1) * 128:SP],
                                         es[:, 128:ncols])

            # view-1 pipeline, tiles 6..0 (DMA order), a_rows descending.
            # First group all-Scalar (same queue = minimum chain latency);
            # later groups split for throughput.
            v1_sq = {6: "scalar", 5: "vector", 4: "vector", 3: "scalar",
                     2: "vector", 1: "scalar", 0: "vector"}
            v1_cvt = {6: "scalar", 4: "scalar"}
            # tiles 6,5 fully per-tile pipelined (first-chain latency):
            # sq5 on DVE runs concurrently with sq6 on Scalar
            square(h1k, 5, 5, eng="vector")
            square(h1k, 6, 6, eng="scalar")
            finish_scale(6, 1)
            transpose_tile(h1k, fT1, 0, 6, cp_engine="scalar")
            a_row(6)
            finish_scale(5, 1)
            transpose_tile(h1k, fT1, 0, 5, cp_engine="vector")
            a_row(5)
            for grp in ((4, 3), (2, 1), (0,)):
                for t in grp:
                    square(h1k, t, t, eng=v1_sq[t])
                finish_scale(min(grp), len(grp))
                for t in grp:
                    transpose_tile(h1k, fT1, 0, t,
                                   cp_engine=v1_cvt.get(t, "vector"))
                for t in grp:
                    a_row(t)

            # view-2 pipeline, tiles 0..6, c_rows as tiles complete
            for grp in ((0, 1), (2, 3), (4, 5), (6,)):
                for t in grp:
                    square(h2k, t, ST + t, eng="scalar")
                finish_scale(ST + grp[0], len(grp))
                for t in grp:
                    transpose_tile(h2k, fT2, 1, t,
                                   cp_engine=("scalar" if t % 2 == 1
                                              else "vector"))
                for t in grp:
                    c_row(t)

            # all view-1 (A + C) folds are in cacp: stage to SBUF, then
            # re-zero behind the first D strip's matmuls
            flush_folds()
            nc.vector.tensor_copy(cacsb1[:], cacp[0:1, :])

            tctx.close()  # free transpose-phase PSUM banks
            ep = bctx.enter_context(tc.tile_pool(name="ep", bufs=1))
            epp = bctx.enter_context(
                tc.tile_pool(name="ep_ps", bufs=1, space="PSUM"))
            pcbt = epp.tile([128, 2, ST], F32, name="pcbt")

            def fold_transpose0():
                # cacp row back to token-partitions: K=1 matmuls
                for jb in range(ST):
                    nc.tensor.matmul(
                        pcbt[:, 0, jb:jb + 1],
                        cacsb1[0:1, jb * 128:(jb + 1) * 128],
                        ones_col[0:1, :], start=True, stop=True,
                        skip_group_check=True)

            def fold_transpose1():
                # cac2 block col-sums: full-K matmuls, N=1 (fast loads)
                for jb in range(ST):
                    nc.tensor.matmul(
                        pcbt[:, 1, jb:jb + 1],
                        cac2[:, jb * 128:(jb + 1) * 128],
                        ones_bf[:], start=True, stop=True,
                        skip_group_check=True)

            ng = ep.tile([128, NB], F32)
            denom = ep.tile([128, NB], F32)
            lg = ep.tile([128, NB], F32)
            ptok = ep.tile([128, NB], F32)
            p20m = ep.tile([128, ST], F32)
            tsum = ep.tile([128, 2], F32)

            def epilogue_half(half):
                """per_tok for one view half -> tsum[:, half]."""
                o = half * ST
                nc.vector.tensor_reduce(ng[:, o:o + ST],
                                        acc[:, o:o + ST, :],
                                        axis=mybir.AxisListType.X,
                                        op=ALU.add)
                nc.vector.tensor_add(ng[:, o:o + ST], ng[:, o:o + ST],
                                     pcbt[:, half, :])
                # Ln(ng + negK0): K0 correction fused into the bias
                nc.scalar.activation(lg[:, o:o + ST], ng[:, o:o + ST],
                                     AF.Ln, bias=negK0[:, 0:1])
                nc.vector.tensor_mul(ptok[:, o:o + ST], lg[:, o:o + ST],
                                     msk24[:, o:o + ST])
                nc.vector.tensor_sub(ptok[:, o:o + ST], ptok[:, o:o + ST],
                                     p20m[:])
                nc.vector.tensor_reduce(tsum[:, half:half + 1],
                                        ptok[:, o:o + ST],
                                        axis=mybir.AxisListType.X,
                                        op=ALU.add)

            # D rows, biggest first so the tail chain hangs off a tiny
            # strip; view-1 epilogue rides along behind the first strips
            d_row(0)
            fold_transpose0()
            nc.vector.tensor_mul(p20m[:], poss20[:], msk[:])
            # poss20 held 64*pos_sim (raw psum); scale to pos_sim/T
            nc.vector.tensor_scalar_mul(p20m[:], p20m[:], EXP_SCALE)
            d_row(1)
            epilogue_half(0)
            for rT in range(2, ST):
                d_row(rT)
            fold_transpose1()
            epilogue_half(1)

            if debug_dump:
                nc.sync.dma_start(ng_dump[:], ng[:])
                nc.sync.dma_start(poss_dump[:], poss20[:])
                nc.sync.dma_start(sc8_dump[:], sc8[:])
            nc.sync.dma_start(out[:], tsum[:])

    return nc


_NC = None


def _stage_core(h1_b, h2_b, mask_b):
    """Host-side compaction: gather unmasked rows, pad to SP, tile, bf16."""
    import ml_dtypes

    idx = np.flatnonzero(mask_b)
    n = idx.size
    if n == 0 or n > SP:
        return None  # numpy fallback handles the (never-seen) extremes
    idxp = np.concatenate(
        [idx, np.full(SP - n, idx[0], dtype=idx.dtype)])
    cmask = (np.arange(SP) < n).astype(np.float32)

    def prep(h):
        hg = h[idxp] * np.float32(4.0)                  # [SP, D], 4h
        hgT = hg.reshape(ST, 128, D).transpose(1, 0, 2)  # [128, ST, D]
        return np.ascontiguousarray(hgT.astype(ml_dtypes.float8_e4m3fn))

    return {
        "hg1": prep(h1_b),
        "hg2": prep(h2_b),
        "cmaskT": np.ascontiguousarray(
            cmask.reshape(ST, 128).T.astype(np.float32)),
    }


def _loss_numpy(h1_b, h2_b, mask_b):
    """Exact reference loss for one sample (fallback, never hit for the
    spec'd mask distribution)."""
    T, EPS = 0.05, 1e-12
    m = mask_b.astype(bool)

    def norm(x):
        nn = np.sqrt((x * x).sum(-1, keepdims=True))
        return x / np.maximum(nn, EPS)

    f1, f2 = norm(h1_b.astype(np.float64)), norm(h2_b.astype(np.float64))
    feats = np.concatenate([f1, f2], 0)
    pos = np.exp((f1 * f2).sum(-1) / T)
    pos = np.concatenate([pos, pos])
    sim = feats @ feats.T / T
    S = h1_b.shape[0]
    tok = np.arange(2 * S) % S
    m2 = np.concatenate([m, m])
    negm = m2[:, None] & m2[None, :] & (tok[:, None] != tok[None, :])
    Ng = (np.exp(sim) * negm).sum(-1)
    per_tok = -np.log(pos / (Ng + pos))
    return float((per_tok * m2).sum() / m2.sum())


def kernel(last_hidden_states_1, last_hidden_states_2, token_mask_batch):
    global _NC
    h1 = np.asarray(last_hidden_states_1, dtype=np.float32)
    h2 = np.asarray(last_hidden_states_2, dtype=np.float32)
    mask = np.asarray(token_mask_batch)
    assert h1.shape == (NCORES, S_FULL, D), h1.shape

    staged, fallback, ns = [], {}, []
    for b in range(NCORES):
        s = _stage_core(h1[b], h2[b], mask[b])
        ns.append(int(mask[b].sum()))
        if s is None:
            fallback[b] = _loss_numpy(h1[b], h2[b], mask[b])
            ph = np.zeros(S_FULL, dtype=bool)
            ph[:SP] = True
            s = _stage_core(h1[b], h2[b], ph)  # placeholder device run
        staged.append(s)

    if _NC is None:
        _NC = _build(NCORES)

    res = run_bass_kernel_spmd(_NC, staged, list(range(NCORES)))
    losses = [
        fallback.get(b,
                     float(np.asarray(res.results[b]["loss"],
                                      dtype=np.float64).sum()
                           / (2.0 * ns[b])))
        for b in range(NCORES)
    ]
    return np.float32(np.mean(losses))


# revision 87
# speedup vs baseline: 1.1925x; 1.0270x over previous
"""ContraCLM token-level contrastive loss on 8 Trainium2 NeuronCores.

Data-parallel over the batch: core b handles sample b (B=8).
138.3us -> ~75us vs the uncompacted baseline.

Mask compaction: ~50% of tokens are masked out and contribute nothing
to the loss except an exp(0)=1 per masked column (corrected by the K0
term). The host gathers only the unmasked token rows (padded with
duplicates of token idx[0] up to SP=896 = 6.5 sigma for
Binomial(1536, 0.5); an exact numpy fallback covers the impossible
overflow) and ships them as fp8e4 (x4). Quadratic sim work drops to
(896/1536)^2 = 34%; input DMA drops 6x. The fp8/fp8-matmul error
averages out to ~4e-5 on the final scalar vs the 2e-2 gate.

Per core the 2SP x 2SP exp-sim row sums come from three quadrant
families (A = f1 f1^T upper triangle, C = f2 f1^T full rows, D =
f2 f2^T upper triangle; fp8e4 x8 DoubleRow matmuls, K=1024 in 4
double-k groups). Row sums ride the ScalarE activation free-dim
accumulator. Normalization: per-token sum-of-squares (Scalar
Square+accum / DVE mul+reduce), then 8*rsqrt(ss) computed as
exp(-ln(ss)/2 + ln 8) so Ln/Exp stay in the same Scalar activation
table as the sim exps (Sqrt would force 1.3us table reloads). The
scale is folded into the transpose: fT tile = hg^T @ diag(sc8), a
plain fp8 x bf16 matmul per k-tile (diag built by one DVE
identity*scalar op), then a f32->fp8 copy into fT via half-tile PSUM
buffers so the copy of one half overlaps the transposes of the next.

Mirrored lower-triangle / B-quadrant contributions are column sums of
the computed strips: view-1 sums accumulate via ones^T @ es fold
matmuls into a persistent PSUM row (zero-initialized by a
zeros-weight matmul; folds emitted one strip late so the PE never
head-blocks on an exp), transposed back to token-partitions by K=1
matmuls mid-kernel. View-2 sums use DVE adds into SBUF + 7 fast
full-K fold matmuls, keeping the tail chain short.

Self-sim diagonals get -1e9 injected in PSUM before exp (exact zero).
Pad/masked tokens have f=0 (mask folded into sc8), each contributing
exp(0)=1 per column: Ln(rowsum + (2n-2SP)) fuses the correction into
the activation bias. per_tok = log(denom) - pos_sim/T; the device
returns per-partition masked sums [128, 2] and the host finishes
sum/(2n) and the 8-core mean.

Schedule: zero-writing keepalive matmuls warm the PE p-state through
the DMA-paced start; input DMAs all on the sync queue (view 1
reversed, in 2-3-tile chunks) so view 1 strictly precedes view 2;
a_rows descend as fT1 tiles complete, c_rows ascend as fT2 tiles
land, d_rows run biggest-first with the view-1 epilogue riding along.
"""

import sys

for _p in ("/opt/trn_rl_repo", "/opt/pypackages"):
    if _p not in sys.path:
        sys.path.append(_p)

from contextlib import ExitStack

import numpy as np

import bass_rust

import concourse.bass as bass
import concourse.tile as tile
from concourse import mybir
from concourse.bass_utils import run_bass_kernel_spmd
from concourse.masks import make_identity
from concourse.vector_clock import ScopedClock

# The walrus build in this container encodes at most 2 sync waits per
# instruction (bass_rust's inst_waits_full agrees), but Tile's semaphore
# assignment can attach more. Hoist excess waits onto unfusable same-engine
# NoOps immediately before the instruction — the engine executes its queue
# in order, so semantics are preserved.
_MAX_WAITS = 1


def _split_excess_waits(nc, ordered):
    for bb_name, insts in ordered.items():
        out = []
        changed = False
        for inst in insts:
            si = getattr(inst, "sync_info", None)
            waits = list(si.on_wait) if si is not None else []
            if len(waits) > _MAX_WAITS:
                changed = True
                extra, keep = waits[:-_MAX_WAITS], waits[-_MAX_WAITS:]
                for i in range(0, len(extra), _MAX_WAITS):
                    out.append(mybir.InstNoOp(
                        name=nc.get_next_instruction_name(),
                        sync_info=mybir.SyncInfo(
                            on_wait=extra[i:i + _MAX_WAITS], on_update=[]),
                        bass_nofuse=True,
                        engine=inst.engine,
                    ))
                si.on_wait = keep
            out.append(inst)
        if changed:
            insts[:] = out


_orig_lower_ordered_insts = tile.TileContext._lower_ordered_insts


def _patched_lower_ordered_insts(self, ordered):
    _split_excess_waits(self.nc, ordered)
    return _orig_lower_ordered_insts(self, ordered)


tile.TileContext._lower_ordered_insts = _patched_lower_ordered_insts


def _split_waits_drain_and_barrier(self, tick_clock, wait_clock):
    nc = self.nc
    probe = nc.sync.nop(nofuse=True)
    wait_clock.add_sem_waits(
        probe.ins, ScopedClock({None: tick_clock.global_clock}))
    si = probe.ins.sync_info
    waits = list(si.on_wait) if si is not None else []
    if len(waits) > _MAX_WAITS:
        si.on_wait = waits[:_MAX_WAITS]
        for i in range(_MAX_WAITS, len(waits), _MAX_WAITS):
            nxt = nc.sync.nop(nofuse=True)
            nxt.ins.sync_info = bass_rust.SyncInfo(
                on_wait=waits[i:i + _MAX_WAITS], on_update=[])
    nc.sync.drain()
    nc.all_engine_barrier()
    assert self.sems is not None
    popped = nc._tile_sem_poison_stack.pop()
    assert popped is self._sem_poison
    nc.clear_and_free_semaphores(list(self.sems.allocated().values()))
    nc.all_engine_barrier()


tile.TileContext._drain_and_barrier = _split_waits_drain_and_barrier

S_FULL, D, NCORES = 1536, 1024, 8
SP = 896                 # compacted+padded tokens per view on device
                         # (Binomial(1536,.5) tops out ~802 for any sane
                         # draw; 896 is 6.5 sigma, and the numpy fallback
                         # is exact if ever exceeded)
ST = SP // 128           # 7 s-tiles per view
NB = 2 * ST              # 16 block rows of F
KT = D // 128            # 8 contraction tiles
TEMP_INV = 20.0          # 1 / 0.05
FP8_SCALE = 8.0          # f entries ~N(0, 1/32); x8 keeps them in e4m3's
                         # normal range (|f|*8 <~ 2, well under 240)
EXP_SCALE = TEMP_INV / (FP8_SCALE * FP8_SCALE)
F32 = mybir.dt.float32
BF16 = mybir.dt.bfloat16
FP8 = mybir.dt.float8e4
AF = mybir.ActivationFunctionType
ALU = mybir.AluOpType
DR = mybir.MatmulPerfMode.DoubleRow


def _chunks(lo, hi):
    """Split [lo, hi) at the PSUM 512-f32 bank boundary."""
    out = []
    if lo < 512:
        out.append((lo, min(512, hi)))
    if hi > 512:
        out.append((max(lo, 512), hi))
    return out


def _build(num_devices: int = NCORES, debug_dump: bool = False) -> bass.Bass:
    nc = bass.Bass(num_devices=num_devices)
    # pre-gathered compacted tokens, [128, ST, D] bf16:
    # partition p, tile t <-> compacted token 128*t + p
    hg1 = nc.dram_tensor("hg1", [128, ST, D], FP8, kind="ExternalInput")
    hg2 = nc.dram_tensor("hg2", [128, ST, D], FP8, kind="ExternalInput")
    cmaskT = nc.dram_tensor("cmaskT", [128, ST], F32, kind="ExternalInput")
    # per-partition per-view token sums; host finishes sum/(2n) + batch mean
    out = nc.dram_tensor("loss", [128, 2], F32, kind="ExternalOutput")
    if debug_dump:
        ng_dump = nc.dram_tensor("ng_dump", [128, NB], F32,
                                 kind="ExternalOutput")
        poss_dump = nc.dram_tensor("poss_dump", [128, ST], F32,
                                   kind="ExternalOutput")
        sc8_dump = nc.dram_tensor("sc8_dump", [128, NB], F32,
                                  kind="ExternalOutput")

    with tile.TileContext(nc) as tc, ExitStack() as ctx:
        const_pool = ctx.enter_context(tc.tile_pool(name="const", bufs=1))
        big = ctx.enter_context(tc.tile_pool(name="big", bufs=1))
        stat = ctx.enter_context(tc.tile_pool(name="stat", bufs=1))

        h1k = big.tile([128, ST, D], FP8)        # 4*h, fp8e4 (host staged)
        h2k = big.tile([128, ST, D], FP8)
        fT1 = big.tile([128, KT, SP], FP8)       # f1^T * 8, fp8e4
        fT2 = big.tile([128, KT, SP], FP8)       # f2^T * 8

        msk = const_pool.tile([128, ST], F32)
        # input DMAs first, all on the sync queue so view 1 gets strict
        # bandwidth priority over view 2 (2-tile chunks, view 1 reversed
        # to match the descending a_row schedule)
        nc.scalar.dma_start(msk[:], cmaskT[:])  # off the h1 critical queue
        for lo, hi in ((5, 7), (2, 5), (0, 2)):
            nc.sync.dma_start(h1k[:, lo:hi, :], hg1[:, lo:hi, :])
        for lo, hi in ((0, 2), (2, 4), (4, 7)):
            nc.sync.dma_start(h2k[:, lo:hi, :], hg2[:, lo:hi, :])

        # keepalive inputs first: PE warmup gates on these two memsets
        zeros_bf = const_pool.tile([128, 1], BF16)
        nc.gpsimd.memset(zeros_bf[:], 0.0)
        ones_b512 = const_pool.tile([128, 512], BF16)
        nc.gpsimd.memset(ones_b512[:], 1.0)
        lnb8 = const_pool.tile([128, 1], F32)
        nc.gpsimd.memset(lnb8[:], float(np.log(FP8_SCALE)))
        identB = const_pool.tile([128, 128], BF16)
        make_identity(nc, identB[:])
        identF = const_pool.tile([128, 128], F32)
        make_identity(nc, identF[:])
        # -1e9 on the diagonal, bf16: injected into self-sim PSUM blocks
        # via an extra accumulating matmul (identB^T @ negIB = -1e9 I)
        negIB = const_pool.tile([128, 128], BF16)
        nc.gpsimd.memset(negIB[:], 0.0)
        nc.gpsimd.affine_select(
            out=negIB[:], in_=negIB[:], compare_op=ALU.not_equal,
            fill=-1e9, base=0, pattern=[[-1, 128]], channel_multiplier=1)
        ones_col = const_pool.tile([128, 1], F32)
        nc.gpsimd.memset(ones_col[:], 1.0)
        ones_sq = const_pool.tile([128, 128], F32)
        nc.gpsimd.memset(ones_sq[:], 1.0)
        ones_bf = const_pool.tile([128, 1], BF16)
        nc.gpsimd.memset(ones_bf[:], 1.0)

        ss = stat.tile([128, NB], F32)           # per-token sum of squares
        sc8 = stat.tile([128, NB], F32)          # 8 * mask * rsqrt(ss)
        nrm = stat.tile([128, NB], F32)
        acc = stat.tile([128, NB, 2], F32)       # per-strip row sums
        poss20 = stat.tile([128, ST], F32)       # 64 * pos_sim
        msk24 = stat.tile([128, NB], F32)
        negK0 = stat.tile([128, 1], F32)

        sqtr = stat.tile([128, 2, D], BF16)      # square scratch
        cacsb1 = stat.tile([1, SP], F32)         # view-1 col sums, SBUF
        cac2 = stat.tile([128, SP], BF16)        # view-2 col acc (D upper)

        nc.gpsimd.memset(acc[:], 0.0)
        nc.vector.memset(cac2[:], 0.0)

        with ExitStack() as bctx:
            psA = bctx.enter_context(
                tc.tile_pool(name="psA", bufs=2, space="PSUM"))
            esA = bctx.enter_context(tc.tile_pool(name="esA", bufs=5))
            scr = bctx.enter_context(tc.tile_pool(name="scr", bufs=4))
            cacpp = bctx.enter_context(
                tc.tile_pool(name="cacp", bufs=1, space="PSUM"))

            cacp = cacpp.tile([1, SP], F32, name="cacp")

            def pe_keepalive(n):
                # small zero-writing matmuls: keep the PE p-state ramp warm
                # through the DMA-paced start (only safe BEFORE real folds
                # accumulate into cacp)
                for _ in range(n):
                    nc.tensor.matmul(cacp[0:1, 0:128], zeros_bf[:],
                                     ones_b512[:, 0:128], start=True,
                                     stop=True, skip_group_check=True)

            def zero_cacp():
                for lo, hi in _chunks(0, SP):
                    nc.tensor.matmul(cacp[0:1, lo:hi],
                                     zeros_bf[:], ones_b512[:, 0:hi - lo],
                                     start=True, stop=True,
                                     skip_group_check=True)

            pe_keepalive(30)

            # ---- mask-only precomputes (before tps claims its PSUM) ----
            with tc.tile_pool(name="ep0", bufs=1) as ep0, \
                 tc.tile_pool(name="ep0_ps", bufs=1, space="PSUM") as ep0p:
                msum = ep0.tile([128, 1], F32)
                nc.vector.tensor_reduce(msum[:], msk[:],
                                        axis=mybir.AxisListType.X,
                                        op=ALU.add)
                nps = ep0p.tile([128, 1], F32)
                nc.tensor.matmul(nps[:], ones_sq[:], msum[:], start=True,
                                 stop=True)
                # -K0 = 2n - 2*SP
                nc.scalar.activation(negK0[:], nps[:], AF.Copy, scale=2.0,
                                     bias=float(-2 * SP))
                nc.vector.tensor_copy(msk24[:, 0:ST], msk[:])
                nc.vector.tensor_copy(msk24[:, ST:NB], msk[:])

            zero_cacp()
            # more p-state warmers: safe until the first fold (PE queue
            # order), they soak the DMA-paced wait for the first h1 tiles
            pe_keepalive(40)
            zero_cacp()
            tctx = ExitStack()  # transpose-phase PSUM, closed before epilogue
            tps = tctx.enter_context(
                tc.tile_pool(name="tps", bufs=2, space="PSUM"))

            def square(hk, t, col, eng):
                """ss[:, col] = sum_d hk[:, t, :]^2."""
                sq = sqtr[:, col % 2, :]
                if eng == "scalar":
                    nc.scalar.activation(sq, hk[:, t, :], AF.Square,
                                         accum_out=ss[:, col:col + 1])
                else:
                    if eng == "gpsimd":
                        nc.gpsimd.tensor_mul(sq, hk[:, t, :], hk[:, t, :])
                    else:
                        nc.vector.tensor_mul(sq, hk[:, t, :], hk[:, t, :])
                    nc.vector.tensor_reduce(ss[:, col:col + 1], sq,
                                            axis=mybir.AxisListType.X,
                                            op=ALU.add)

            def finish_scale(o, n):
                """sc8[:, o:o+n] = 8 * msk * rsqrt(ss[:, o:o+n]).

                rsqrt as exp(-ln(x)/2 + ln 8): Ln/Exp share the Scalar
                activation table with the sim exps — no table reloads
                (Sqrt would force a 1.3us table swap each way).
                """
                nc.scalar.activation(nrm[:, o:o + n], ss[:, o:o + n],
                                     AF.Ln)
                r8 = stat.tile([128, n], F32, name=f"r8_{o}")
                nc.scalar.activation(r8[:], nrm[:, o:o + n], AF.Exp,
                                     scale=-0.5, bias=lnb8[:, 0:1])
                nc.vector.tensor_mul(sc8[:, o:o + n], r8[:],
                                     msk24[:, o:o + n])

            def transpose_tile(hk, fT, half, t, cp_engine):
                """fT[:, :, t*128:+128] = (hk[:,t,:]/4 * sc8)^T as fp8.

                Scale + transpose in one plain matmul per k-tile:
                out = hk_tile^T @ diag(sc8) (hk is 4h fp8; diag carries
                8*rsqrt(ss8) = 2/||h||, so out = 8*h/||h||).
                """
                col = half * ST + t
                diagS = scr.tile([128, 128], BF16, tag="dg8",
                                 name=f"dgS_{half}_{t}")
                # diag(sc8) = identity * sc8 per-partition: one fast DVE op
                nc.vector.tensor_scalar_mul(diagS[:], identB[:],
                                            sc8[:, col:col + 1])
                # half-tile pt buffers (1 PSUM bank each, 2 in the pool):
                # the cvt of one half overlaps the transposes of the next
                for hf in range(2):
                    k0 = hf * (KT // 2)
                    pt = tps.tile([128, D // 2], F32, tag="pt",
                                  name=f"pt_{half}_{t}_{hf}")
                    for k in range(KT // 2):
                        nc.tensor.matmul(
                            pt[:, k * 128:(k + 1) * 128],
                            hk[:, t, (k0 + k) * 128:(k0 + k + 1) * 128],
                            diagS[:], start=True, stop=True)
                    dst = fT[:, k0:k0 + KT // 2, t * 128:(t + 1) * 128]
                    src = pt[:].rearrange("p (j c) -> p j c", j=KT // 2)
                    if cp_engine == "vector":
                        nc.vector.tensor_copy(dst, src)
                    else:
                        nc.scalar.copy(dst, src)

            def mm_strip(ps, lhsT, rT, rhsT, col0, ncols):
                """sim strip into ps[:, 0:ncols] (DoubleRow, K=1024)."""
                for g in range(KT // 2):
                    u0 = 0
                    while u0 < ncols:
                        u1 = min(u0 + 512, ncols)
                        nc.tensor.matmul(
                            ps[:, u0:u1],
                            lhsT[:, 2 * g:2 * g + 2,
                                 rT * 128:(rT + 1) * 128],
                            rhsT[:, 2 * g:2 * g + 2, col0 + u0:col0 + u1],
                            perf_mode=DR,
                            start=(g == 0), stop=(g == KT // 2 - 1))
                        u0 = u1

            # deferred column folds: emitted after the NEXT strip's matmuls
            # so the PE queue never head-blocks on an exp result
            pending_folds = []

            def flush_folds():
                for g0, g1, es_ap in pending_folds:
                    for lo, hi in _chunks(g0, g1):
                        nc.tensor.matmul(
                            cacp[0:1, lo:hi], ones_bf[:],
                            es_ap[:, lo - g0:hi - g0],
                            start=False, stop=True, skip_group_check=True)
                pending_folds.clear()

            def a_row(r):
                ncols = SP - r * 128
                trip = psA.tile([128, SP], F32, tag="tp", name=f"tpA_{r}")
                mm_strip(trip, fT1, r, fT1, r * 128, ncols)
                flush_folds()
                nc.tensor.matmul(trip[:, 0:128], identB[:], negIB[:],
                                 start=False, stop=True,
                                 skip_group_check=True)
                es = esA.tile([128, SP], BF16, tag="es", name=f"esA_{r}")
                nc.scalar.activation(es[:, 0:ncols], trip[:, 0:ncols],
                                     AF.Exp, scale=EXP_SCALE,
                                     accum_out=acc[:, r, 0:1])
                if ncols > 128:
                    pending_folds.append(
                        ((r + 1) * 128, SP, es[:, 128:ncols]))

            def c_row(rT):
                trip = psA.tile([128, SP], F32, tag="tp", name=f"tpC_{rT}")
                mm_strip(trip, fT2, rT, fT1, 0, SP)
                flush_folds()
                # counterpart diagonal: extract 64*pos_sim, keep it inside
                # the row sum (denom = Ng + pos)
                dscr = scr.tile([128, 128], F32, tag="dg", name=f"dg_{rT}")
                nc.vector.tensor_mul(
                    dscr[:], trip[:, rT * 128:(rT + 1) * 128], identF[:])
                nc.vector.tensor_reduce(
                    poss20[:, rT:rT + 1], dscr[:],
                    axis=mybir.AxisListType.X, op=ALU.add)
                es = esA.tile([128, SP], BF16, tag="es", name=f"esC_{rT}")
                nc.scalar.activation(es[:], trip[:], AF.Exp,
                                     scale=EXP_SCALE,
                                     accum_out=acc[:, ST + rT, 0:1])
                pending_folds.append((0, SP, es[:]))

            def d_row(rT):
                ncols = SP - rT * 128
                trip = psA.tile([128, SP], F32, tag="tp", name=f"tpD_{rT}")
                mm_strip(trip, fT2, rT, fT2, rT * 128, ncols)
                flush_folds()
                nc.tensor.matmul(trip[:, 0:128], identB[:], negIB[:],
                                 start=False, stop=True,
                                 skip_group_check=True)
                es = esA.tile([128, SP], BF16, tag="es", name=f"esD_{rT}")
                nc.scalar.activation(es[:, 0:ncols], trip[:, 0:ncols],
                                     AF.Exp, scale=EXP_SCALE,
                                     accum_out=acc[:, ST + rT, 1:2])
                if ncols > 128:
                    # view-2 col sums on DVE (bf16 2x) — the final fold
                    # back to token-partitions is then 7 fast full-K
                    # matmuls instead of slow K=1 loads on the tail
                    nc.vector.tensor_add(cac2[:, (rT + 1) * 128:SP],
                                         cac2[:, (rT + 1) * 128:SP],
                                         es[:, 128:ncols])

            # view-1 pipeline, tiles 6..0 (DMA order), a_rows descending.
            # First group all-Scalar (same queue = minimum chain latency);
            # later groups split for throughput.
            v1_sq = {6: "scalar", 5: "vector", 4: "vector", 3: "scalar",
                     2: "vector", 1: "scalar", 0: "vector"}
            v1_cvt = {6: "scalar", 4: "scalar"}
            # tiles 6,5 fully per-tile pipelined (first-chain latency):
            # both squares on Scalar so the DVE queue stays clear for the
            # sc8 -> diagS chain; cvts on the free DVE
            square(h1k, 6, 6, eng="scalar")
            square(h1k, 5, 5, eng="scalar")
            finish_scale(6, 1)
            transpose_tile(h1k, fT1, 0, 6, cp_engine="vector")
            a_row(6)
            finish_scale(5, 1)
            transpose_tile(h1k, fT1, 0, 5, cp_engine="vector")
            a_row(5)
            for grp in ((4, 3), (2, 1), (0,)):
                for t in grp:
                    square(h1k, t, t, eng=v1_sq[t])
                finish_scale(min(grp), len(grp))
                for t in grp:
                    transpose_tile(h1k, fT1, 0, t,
                                   cp_engine=v1_cvt.get(t, "vector"))
                for t in grp:
                    a_row(t)

            # view-2 pipeline, tiles 0..6, c_rows as tiles complete
            for grp in ((0, 1), (2, 3), (4, 5), (6,)):
                for t in grp:
                    square(h2k, t, ST + t, eng="scalar")
                finish_scale(ST + grp[0], len(grp))
                for t in grp:
                    transpose_tile(h2k, fT2, 1, t,
                                   cp_engine=("scalar" if t % 2 == 1
                                              else "vector"))
                for t in grp:
                    c_row(t)

            # all view-1 (A + C) folds are in cacp: stage to SBUF, then
            # re-zero behind the first D strip's matmuls
            flush_folds()
            nc.vector.tensor_copy(cacsb1[:], cacp[0:1, :])

            tctx.close()  # free transpose-phase PSUM banks
            ep = bctx.enter_context(tc.tile_pool(name="ep", bufs=1))
            epp = bctx.enter_context(
                tc.tile_pool(name="ep_ps", bufs=1, space="PSUM"))
            pcbt = epp.tile([128, 2, ST], F32, name="pcbt")

            def fold_transpose0():
                # cacp row back to token-partitions: K=1 matmuls
                for jb in range(ST):
                    nc.tensor.matmul(
                        pcbt[:, 0, jb:jb + 1],
                        cacsb1[0:1, jb * 128:(jb + 1) * 128],
                        ones_col[0:1, :], start=True, stop=True,
                        skip_group_check=True)

            def fold_transpose1():
                # cac2 block col-sums: full-K matmuls, N=1 (fast loads)
                for jb in range(ST):
                    nc.tensor.matmul(
                        pcbt[:, 1, jb:jb + 1],
                        cac2[:, jb * 128:(jb + 1) * 128],
                        ones_bf[:], start=True, stop=True,
                        skip_group_check=True)

            ng = ep.tile([128, NB], F32)
            denom = ep.tile([128, NB], F32)
            lg = ep.tile([128, NB], F32)
            ptok = ep.tile([128, NB], F32)
            p20m = ep.tile([128, ST], F32)
            tsum = ep.tile([128, 2], F32)

            def epilogue_half(half):
                """per_tok for one view half -> tsum[:, half]."""
                o = half * ST
                nc.vector.tensor_reduce(ng[:, o:o + ST],
                                        acc[:, o:o + ST, :],
                                        axis=mybir.AxisListType.X,
                                        op=ALU.add)
                nc.vector.tensor_add(ng[:, o:o + ST], ng[:, o:o + ST],
                                     pcbt[:, half, :])
                # Ln(ng + negK0): K0 correction fused into the bias
                nc.scalar.activation(lg[:, o:o + ST], ng[:, o:o + ST],
                                     AF.Ln, bias=negK0[:, 0:1])
                nc.vector.tensor_mul(ptok[:, o:o + ST], lg[:, o:o + ST],
                                     msk24[:, o:o + ST])
                nc.vector.tensor_sub(ptok[:, o:o + ST], ptok[:, o:o + ST],
                                     p20m[:])
                nc.vector.tensor_reduce(tsum[:, half:half + 1],
                                        ptok[:, o:o + ST],
                                        axis=mybir.AxisListType.X,
                                        op=ALU.add)

            # D rows, biggest first so the tail chain hangs off a tiny
            # strip; view-1 epilogue rides along behind the first strips
            d_row(0)
            fold_transpose0()
            nc.vector.tensor_mul(p20m[:], poss20[:], msk[:])
            # poss20 held 64*pos_sim (raw psum); scale to pos_sim/T
            nc.vector.tensor_scalar_mul(p20m[:], p20m[:], EXP_SCALE)
            d_row(1)
            epilogue_half(0)
            for rT in range(2, ST):
                d_row(rT)
            fold_transpose1()
            epilogue_half(1)

            if debug_dump:
                nc.sync.dma_start(ng_dump[:], ng[:])
                nc.sync.dma_start(poss_dump[:], poss20[:])
                nc.sync.dma_start(sc8_dump[:], sc8[:])
            nc.sync.dma_start(out[:], tsum[:])

    return nc


_NC = None


def _stage_core(h1_b, h2_b, mask_b):
    """Host-side compaction: gather unmasked rows, pad to SP, tile, bf16."""
    import ml_dtypes

    idx = np.flatnonzero(mask_b)
    n = idx.size
    if n == 0 or n > SP:
        return None  # numpy fallback handles the (never-seen) extremes
    idxp = np.concatenate(
        [idx, np.full(SP - n, idx[0], dtype=idx.dtype)])
    cmask = (np.arange(SP) < n).astype(np.float32)

    def prep(h):
        hg = h[idxp] * np.float32(4.0)                  # [SP, D], 4h
        hgT = hg.reshape(ST, 128, D).transpose(1, 0, 2)  # [128, ST, D]
        return np.ascontiguousarray(hgT.astype(ml_dtypes.float8_e4m3fn))

    return {
        "hg1": prep(h1_b),
        "hg2": prep(h2_b),
        "cmaskT": np.ascontiguousarray(
            cmask.reshape(ST, 128).T.astype(np.float32)),
    }


def _loss_numpy(h1_b, h2_b, mask_b):
    """Exact reference loss for one sample (fallback, never hit for the
    spec'd mask distribution)."""
    T, EPS = 0.05, 1e-12
    m = mask_b.astype(bool)

    def norm(x):
        nn = np.sqrt((x * x).sum(-1, keepdims=True))
        return x / np.maximum(nn, EPS)

    f1, f2 = norm(h1_b.astype(np.float64)), norm(h2_b.astype(np.float64))
    feats = np.concatenate([f1, f2], 0)
    pos = np.exp((f1 * f2).sum(-1) / T)
    pos = np.concatenate([pos, pos])
    sim = feats @ feats.T / T
    S = h1_b.shape[0]
    tok = np.arange(2 * S) % S
    m2 = np.concatenate([m, m])
    negm = m2[:, None] & m2[None, :] & (tok[:, None] != tok[None, :])
    Ng = (np.exp(sim) * negm).sum(-1)
    per_tok = -np.log(pos / (Ng + pos))
    return float((per_tok * m2).sum() / m2.sum())


def kernel(last_hidden_states_1, last_hidden_states_2, token_mask_batch):
    global _NC
    h1 = np.asarray(last_hidden_states_1, dtype=np.float32)
    h2 = np.asarray(last_hidden_states_2, dtype=np.float32)
    mask = np.asarray(token_mask_batch)
    assert h1.shape == (NCORES, S_FULL, D), h1.shape

    staged, fallback, ns = [], {}, []
    for b in range(NCORES):
        s = _stage_core(h1[b], h2[b], mask[b])
        ns.append(int(mask[b].sum()))
        if s is None:
            fallback[b] = _loss_numpy(h1[b], h2[b], mask[b])
            ph = np.zeros(S_FULL, dtype=bool)
            ph[:SP] = True
            s = _stage_core(h1[b], h2[b], ph)  # placeholder device run
        staged.append(s)

    if _NC is None:
        _NC = _build(NCORES)

    res = run_bass_kernel_spmd(_NC, staged, list(range(NCORES)))
    losses = [
        fallback.get(b,
                     float(np.asarray(res.results[b]["loss"],
                                      dtype=np.float64).sum()
                           / (2.0 * ns[b])))
        for b in range(NCORES)
    ]
    return np.float32(np.mean(losses))
